# revision 20
# baseline (speedup 1.0000x reference)
"""Causal self-attention on 8 Trainium2 NeuronCores (Bass/Tile).

Problem: B=4, S=2048, E=1024, H=16 heads, D=64.
  y = softmax(causal(Q K^T / sqrt(D))) V @ w_proj

Sharding: tensor-parallel over heads. Core c owns heads (2c, 2c+1): it computes
the Q/K/V projections for its 384 columns of w_att, runs attention for its two
heads, and multiplies by its 128-row slice of w_proj, producing a full-shape
partial output. Partials are summed on the host (the all-reduce of the
row-sharded c_proj).

All matmul operands are bf16 (fp32 PSUM accumulation; measured ~4e-3 rel err
end-to-end vs the 2e-2 gate). bf16 enables the PE fast-weight-load path (FWL
needs a non-fp32 dtype) and halves DMA/SBUF traffic. The softmax 1/sqrt(D) is
folded into wq on the host.

Per-core dataflow (feature-on-partition "T" layouts for Q/K/Y):
  phase A: QT/KT [128(2h x 64d), S] = w_slice.T @ xT (x passed pre-transposed);
           V built natural-layout [t, d] directly: psv = x_slice.T @ wv
           (8 accumulating K=128 matmuls, N=128), so no PE transpose is needed;
           a ones column is appended per head (softmax denominator) and padded
           to 128 so the PV matmul has a valid 128-partition destination.
  phase B: St [128k, 512q] = KT_slice.T @ QT_slice (contraction d=64; the two
           heads land in different PE row-groups -> row-tiled concurrent
           matmuls sharing one [128, 1024] two-bank PSUM tile); Pt = exp(St)
           -- one activation per k-tile (diagonal tiles use a 3D AP covering
           both heads' partial ranges); no max-shift needed since logits are
           ~N(0,1); causality: k>q tiles never computed, diagonal-region
           columns excluded from matmul/exp ranges, one 3D-AP affine_select
           masks both heads' diagonal 128-blocks; Y^T and the softmax
           denominator come from one matmul per head:
           lhsT=[V|1|0] [128k, 128], rhs=Pt -> psum [128, 512] accumulated
           over k.
  phase C: out[t, :] = Yt_slice.T @ w_proj_slice with 1/denom folded into Yt
           (denominator reciprocal broadcast across partitions via K=1 matmul
           against a selector); output staged per q-block in bf16 and stored
           with one DMA per 512 tokens.

The build interleaves phases to keep the tensor engine busy: phase A of batch
b+1 and phase C of batch b (eligible stripe-by-stripe as q-blocks normalize)
are emitted as filler units inside phase B of batch b, and PV lags St by two
k-tiles so exp latency is hidden.

PSUM->SBUF copies run on DVE except the small bc copy (ACT), keeping both
engines below the tensor-engine critical path.
"""


from contextlib import ExitStack, nullcontext

import numpy as np

import concourse.bacc as bacc
import concourse.tile as tile
from concourse import mybir

B, S, E, H, D = 4, 2048, 1024, 16, 64
NCORES = 8
HPC = H // NCORES
T = B * S
EC = E // 128
NB = S // 512
NQ = S // 512
NK = S // 128
NT = S // 128
SCALE = 1.0 / np.sqrt(D)

F32 = mybir.dt.float32
MMDT = mybir.dt.bfloat16


def build_nc(repeat: int = 1):
    nc = bacc.Bacc("TRN2", target_bir_lowering=False, debug=False, enable_asserts=False)

    xT_d = nc.dram_tensor("xT", [E, T], MMDT, kind="ExternalInput")
    wqkv_d = nc.dram_tensor("wqkv", [E, 3 * 128], MMDT, kind="ExternalInput")
    wproj_d = nc.dram_tensor("wproj", [128, E], MMDT, kind="ExternalInput")
    out_d = nc.dram_tensor("out", [T, E], MMDT, kind="ExternalOutput")

    xT_v = xT_d.ap().rearrange("(c p) t -> p c t", p=128)
    wqkv_v = wqkv_d.ap().rearrange("(c p) m -> p c m", p=128)
    # [B, NQ, 128(p), 4(ti), E] so one DMA stores a whole q-block (512 tokens)
    # straight out of a partition-first [128, 4, 1024] staging tile
    out_v = out_d.ap().rearrange("(bb qq tt p) o -> bb qq p tt o", bb=B, qq=NQ, p=128)

    with tile.TileContext(nc) as tc, ExitStack() as ctx:
        consts = ctx.enter_context(tc.tile_pool(name="consts", bufs=1))
        weights = ctx.enter_context(tc.tile_pool(name="weights", bufs=1))
        xpool = ctx.enter_context(tc.tile_pool(name="xpool", bufs=4))
        qkv = ctx.enter_context(tc.tile_pool(name="qkv", bufs=2))
        ptpool = ctx.enter_context(tc.tile_pool(name="ptpool", bufs=8))
        smalls = ctx.enter_context(tc.tile_pool(name="smalls", bufs=2))
        opool = ctx.enter_context(tc.tile_pool(name="opool", bufs=3))
        ps_a = ctx.enter_context(tc.tile_pool(name="ps_a", bufs=2, space="PSUM"))
        ps_s = ctx.enter_context(tc.tile_pool(name="ps_s", bufs=2, space="PSUM"))
        ps_y = ctx.enter_context(tc.tile_pool(name="ps_y", bufs=1, space="PSUM"))

        ones16 = consts.tile([128, 16], MMDT)
        nc.gpsimd.memset(ones16[:], 1.0)
        # head-selector for the denominator broadcast matmul: bc = sel.T @ r01
        # (rows 0 and 32 hold the two heads' recips; other rows are zero)
        sel = consts.tile([64, 128], MMDT)
        nc.gpsimd.memset(sel[:], 0.0)
        nc.gpsimd.memset(sel[0:1, 0:64], 1.0)
        nc.gpsimd.memset(sel[32:33, 64:128], 1.0)
        r01 = consts.tile([64, 512], MMDT)
        nc.gpsimd.memset(r01[:], 0.0)

        wqkv_sb = weights.tile([128, EC, 3 * 128], MMDT)
        nc.sync.dma_start(wqkv_sb[:], wqkv_v)
        wproj_sb = weights.tile([128, E], MMDT)
        nc.sync.dma_start(wproj_sb[:], wproj_d.ap())

        # per-batch persistent tiles, allocated lazily
        tiles = {}

        def get_tiles(b):
            if b not in tiles:
                tiles[b] = {
                    "QT": qkv.tile([128, S], MMDT, tag="QT", name="QT"),
                    "KT": qkv.tile([128, S], MMDT, tag="KT", name="KT"),
                    "V": qkv.tile([128, NK, 256], MMDT, tag="V", name="V"),
                    "Yt": qkv.tile([128, S], MMDT, tag="Yt", name="Yt"),
                }
            return tiles[b]

        def phase_a_units(b):
            """Yield closures, each emitting one chunk of phase A for batch b."""
            tl = get_tiles(b)
            QT, KT, V = tl["QT"], tl["KT"], tl["V"]
            tb = b * S

            def ones_cols():
                nc.vector.tensor_copy(V[:, :, 64], ones16[:])
                nc.vector.tensor_copy(V[:, :, 192], ones16[:])
                nc.gpsimd.memset(V[:, :, 65:128], 0.0)
                nc.gpsimd.memset(V[:, :, 193:256], 0.0)

            boxes = [{} for _ in range(NB)]

            def load_x(nb):
                # two half tiles so the first matmuls can start after half
                # the transfer (tile-granular dependency tracking)
                x_a = xpool.tile([128, EC // 2, 512], MMDT, tag="xa", name="x_a")
                x_b = xpool.tile([128, EC // 2, 512], MMDT, tag="xb", name="x_b")
                t0 = tb + nb * 512
                nc.sync.dma_start(x_a[:], xT_v[:, 0 : EC // 2, t0 : t0 + 512])
                nc.sync.dma_start(x_b[:], xT_v[:, EC // 2 : EC, t0 : t0 + 512])
                boxes[nb]["t"] = (x_a, x_b)

            def ones_and_loads():
                ones_cols()
                load_x(0)
                load_x(1)

            ones_and_loads.pe_cost = 0.1
            yield ones_and_loads
            for nb in range(NB):
                xt_box = boxes[nb]

                def xc(xt_box, ec):
                    return xt_box["t"][ec // (EC // 2)][:, ec % (EC // 2), :]

                def q_group(nb=nb, xt_box=xt_box):
                    if nb + 2 < NB:
                        load_x(nb + 2)
                    psq = ps_a.tile([128, 512], F32, tag="A")
                    for ec in range(EC):
                        nc.tensor.matmul(
                            psq[:], wqkv_sb[:, ec, 0:128], xc(xt_box, ec),
                            start=(ec == 0), stop=(ec == EC - 1),
                        )
                    nc.vector.tensor_copy(QT[:, nb * 512 : (nb + 1) * 512], psq[:])

                def k_group(nb=nb, xt_box=xt_box):
                    psk = ps_a.tile([128, 512], F32, tag="A")
                    for ec in range(EC):
                        nc.tensor.matmul(
                            psk[:], wqkv_sb[:, ec, 128:256], xc(xt_box, ec),
                            start=(ec == 0), stop=(ec == EC - 1),
                        )
                    nc.vector.tensor_copy(KT[:, nb * 512 : (nb + 1) * 512], psk[:])

                def v_unit(j, nb=nb, xt_box=xt_box):
                    # natural-layout V for one 128-token tile: [t, 2h x 64d]
                    psv = ps_a.tile([128, 512], F32, tag="A")
                    for ec in range(EC):
                        nc.tensor.matmul(
                            psv[:, 0:128],
                            xc(xt_box, ec)[:, j * 128 : (j + 1) * 128],
                            wqkv_sb[:, ec, 256:384],
                            start=(ec == 0), stop=(ec == EC - 1),
                        )
                    st = nb * 4 + j
                    dst = V[:, st].rearrange("p (g c) -> p g c", g=2)[:, :, 0:64]
                    src = psv[:, 0:128].rearrange("p (g c) -> p g c", g=2)
                    nc.vector.tensor_copy(dst, src)

                q_group.pe_cost = 1.7
                k_group.pe_cost = 1.7
                yield q_group
                yield k_group
                for j in range(4):
                    vu = lambda j=j, nb=nb, xb=xt_box: v_unit(j, nb, xb)
                    vu.pe_cost = 0.45
                    yield vu

        def phase_c_units(b, qb):
            """Output projection for q-block qb of batch b: 4 token tiles +
            one batched DMA of the 512-token stripe."""
            tl = get_tiles(b)
            Yt = tl["Yt"]
            box = {}
            units = []

            def c_unit(i, ti, Yt=Yt, b=b):
                if i == 0:
                    box["o"] = opool.tile([128, 4, 1024], MMDT, tag="osb", name="o_sb")
                o_sb = box["o"]
                for oh in range(2):
                    pso = ps_a.tile([128, 512], F32, tag="A")
                    nc.tensor.matmul(
                        pso[:],
                        Yt[:, ti * 128 : (ti + 1) * 128],
                        wproj_sb[:, oh * 512 : (oh + 1) * 512],
                        start=True, stop=True,
                    )
                    if tail_mode[0] and oh == 1:
                        # after the last exp, ACT is idle: split the PSUM
                        # evacuation across engines to shorten the tail
                        nc.scalar.copy(o_sb[:, i, oh * 512 : (oh + 1) * 512], pso[:])
                    else:
                        nc.vector.tensor_copy(
                            o_sb[:, i, oh * 512 : (oh + 1) * 512], pso[:]
                        )

            def store(b=b, qb=qb):
                # Pool-engine DGE: keeps output stores off the SP queue that
                # carries the x loads (a shared queue lets a store deadlock
                # behind a blocked x load via the staging-pool WAR chain)
                o_sb = box["o"]
                nc.gpsimd.dma_start(out_v[b, qb], o_sb[:])

            for i, ti in enumerate(range(4 * qb, 4 * qb + 4)):
                cu = lambda i=i, ti=ti: c_unit(i, ti)
                cu.pe_cost = 0.45
                units.append(cu)
            store.pe_cost = 0.0
            units.append(store)
            return units

        ready_units = []  # FIFO of pending filler closures
        tail_mode = [False]  # set once all exps are emitted (ACT near-idle)

        def fill(min_cost=0.85):
            # pop units until ~min_cost us of PE work is queued, so the
            # filler actually covers the stall it was placed in front of
            got = 0.0
            while ready_units and got < min_cost:
                u = ready_units.pop(0)
                got += getattr(u, "pe_cost", 1.0)
                u()

        def phase_b(b):
            """Attention for batch b; drains ready_units at a steady cadence.

            Each q-block's finalize (reciprocal -> bc broadcast -> Yt
            normalize -> phase-C units) is deferred into the filler FIFO so a
            couple of the next q-block's S matmuls queue ahead of the bc
            matmul, hiding the reciprocal latency from the tensor engine. The
            psy allocation happens at the first PV so the deferred reads of
            the previous incarnation are emitted before the ring reuses it.
            """
            tl = get_tiles(b)
            QT, KT, V, Yt = tl["QT"], tl["KT"], tl["V"], tl["Yt"]
            for qb in range(NQ):
                q0 = qb * 512
                nkj = 4 * qb + 4
                ybox = {}
                pend = []  # (kj, pt0, pt1, z) awaiting PV

                def emit_pv(kj, pt, z, ybox=ybox, nkj=nkj):
                    if kj == 0:
                        ybox["y0"] = ps_y.tile([128, 512], F32, tag="y0", name="psy0")
                        ybox["y1"] = ps_y.tile([128, 512], F32, tag="y1", name="psy1")
                    psy0, psy1 = ybox["y0"], ybox["y1"]
                    nc.tensor.matmul(
                        psy0[:, z:512], V[:, kj, 0:128], pt[:, z:512],
                        start=(kj == 0), stop=(kj == nkj - 1),
                    )
                    nc.tensor.matmul(
                        psy1[:, z:512], V[:, kj, 128:256], pt[:, 512 + z : 1024],
                        start=(kj == 0), stop=(kj == nkj - 1),
                    )

                for kj in range(nkj):
                    fill()
                    z = max(0, (kj - 4 * qb) * 128)
                    pss = ps_s.tile([128, 1024], F32, tag="S", name="pss")
                    ks = slice(kj * 128, (kj + 1) * 128)
                    qs = slice(q0 + z, q0 + 512)
                    nc.tensor.matmul(
                        pss[:, z:512], KT[0:64, ks], QT[0:64, qs],
                        start=True, stop=True,
                    )
                    nc.tensor.matmul(
                        pss[:, 512 + z : 1024], KT[64:128, ks], QT[64:128, qs],
                        start=True, stop=True,
                    )
                    pt = ptpool.tile([128, 1024], MMDT, tag="pt", name="pt")
                    if z == 0:  # one activation covers both heads' tiles
                        nc.scalar.activation(
                            pt[:], pss[:], mybir.ActivationFunctionType.Exp
                        )
                    else:  # one 3D-AP activation covers both heads' live ranges
                        nc.scalar.activation(
                            pt[:].rearrange("p (h q) -> p h q", h=2)[:, :, z:512],
                            pss[:].rearrange("p (h q) -> p h q", h=2)[:, :, z:512],
                            mybir.ActivationFunctionType.Exp,
                        )
                    if kj - 4 * qb >= 0:  # diagonal blocks, both heads at once
                        diag = pt[:].rearrange("p (h q) -> p h q", h=2)[:, :, z : z + 128]
                        nc.gpsimd.affine_select(
                            out=diag,
                            in_=diag,
                            compare_op=mybir.AluOpType.is_ge,
                            fill=0.0,
                            base=0,
                            pattern=[[0, 2], [1, 128]],
                            channel_multiplier=-1,
                        )
                    pend.append((kj, pt, z))
                    if len(pend) > 2:  # PV lags St by 2 k-tiles
                        emit_pv(*pend.pop(0))
                fill(0.85)
                while pend:
                    emit_pv(*pend.pop(0))

                def finalize(qb=qb, q0=q0, ybox=ybox, b=b, Yt=Yt):
                    psy0, psy1 = ybox["y0"], ybox["y1"]
                    with nc.allow_low_precision(reason="bf16 recip, bf16 mm"):
                        nc.vector.reciprocal(r01[0:1, :], psy0[64:65, :])
                        nc.vector.reciprocal(r01[32:33, :], psy1[64:65, :])
                    ps_bc = ps_a.tile([128, 512], F32, tag="A")
                    nc.tensor.matmul(ps_bc[:], sel[:], r01[:], start=True, stop=True)
                    bc = smalls.tile([128, 512], MMDT, tag="bc")
                    nc.scalar.copy(bc[:], ps_bc[:])
                    with nc.allow_low_precision(reason="bf16 Yt feeds bf16 mm"):
                        nc.vector.tensor_tensor(
                            Yt[0:64, q0 : q0 + 512], psy0[0:64, :], bc[0:64, :],
                            mybir.AluOpType.mult,
                        )
                        nc.vector.tensor_tensor(
                            Yt[64:128, q0 : q0 + 512], psy1[0:64, :], bc[64:128, :],
                            mybir.AluOpType.mult,
                        )
                    # this q-block's output-projection tiles are now computable
                    ready_units.extend(phase_c_units(b, qb))

                # defer: ~2 of the next q-block's S k-tiles queue ahead of it
                ready_units.insert(min(1, len(ready_units)), finalize)

        loop_cm = (
            tc.For_i(0, repeat, 1, hint_engines=tuple(nc.engines))
            if repeat > 1
            else nullcontext()
        )
        with loop_cm:
            # phase A for batch 0 up front
            for u in phase_a_units(0):
                u()
            for b in range(B):
                if b + 1 < B:
                    ready_units.extend(phase_a_units(b + 1))
                phase_b(b)
            tail_mode[0] = True
            while ready_units:
                fill()
            tail_mode[0] = False
            tiles.clear()

    nc.compile()
    return nc


def shard_inputs(x: np.ndarray, w_att: np.ndarray, w_proj: np.ndarray):
    """Full inputs -> 8 per-core input dicts (head-sharded bf16 weights,
    shared bf16 xT; softmax scale folded into wq)."""
    import ml_dtypes

    bf16 = ml_dtypes.bfloat16
    xT = np.ascontiguousarray(
        np.asarray(x, dtype=np.float32).reshape(T, E).T.astype(bf16)
    )
    w_att = np.asarray(w_att, dtype=np.float32)
    w_proj = np.asarray(w_proj, dtype=np.float32)
    wq, wk, wv = w_att[:, :E] * SCALE, w_att[:, E : 2 * E], w_att[:, 2 * E :]
    in_maps = []
    for c in range(NCORES):
        h0 = HPC * c
        cols = []
        for w in (wq, wk, wv):
            cols.append(w[:, h0 * D : (h0 + 1) * D])
            cols.append(w[:, (h0 + 1) * D : (h0 + 2) * D])
        wqkv_c = np.ascontiguousarray(np.concatenate(cols, axis=1).astype(bf16))
        wproj_c = np.ascontiguousarray(w_proj[c * 128 : (c + 1) * 128, :].astype(bf16))
        in_maps.append({"xT": xT, "wqkv": wqkv_c, "wproj": wproj_c})
    return in_maps


_NC_CACHE = {}


def get_nc():
    if "nc" not in _NC_CACHE:
        _NC_CACHE["nc"] = build_nc()
    return _NC_CACHE["nc"]


def kernel(x: np.ndarray, w_att: np.ndarray, w_proj: np.ndarray) -> np.ndarray:
    from concourse.bass_utils import run_bass_kernel_spmd

    nc = get_nc()
    in_maps = shard_inputs(x, w_att, w_proj)
    res = run_bass_kernel_spmd(nc, in_maps, core_ids=list(range(NCORES)))
    acc = res.results[0]["out"].astype(np.float32)
    for r in res.results[1:]:
        acc += r["out"].astype(np.float32)
    return acc.reshape(B, S, E)
